# revision 21
# baseline (speedup 1.0000x reference)
"""AdaAtt attention block on 8 TRN2 NeuronCores — v4.

Data-parallel over batch (16/core), weights replicated.

v3 trace (106us) postmortem: the batch loop and the softmax/visAtt loop
were separate, so on the strict-FIFO ACT/DVE queues every softmax op
queued behind all 16 tanh instructions (g0's reciprocal ran 33us after
its inputs were ready) and the whole visAtt+tail serialized after the
batch phase. tensor_scalar_add also ran in 1x mode (the scalar operand
is forced f32, which disqualifies the packed modes), so the fp8->bf16
cast-DMA bought nothing and its 6.4MB of SBUF writes slowed the weight
stream.

v4:
- Softmax + pi transpose + masked visAtt MMs interleaved into the batch
  loop right after each group's last score matmul.
- Adds are plain tensor_tensor (fp8 cfe + stride-0 hoe broadcast, 1x) at
  ~1.7us/batch, with 4 of 16 batches on the otherwise-idle GpSimd; cfe
  loads move back to the sync HWDGE queue (no cast, no SWDGE).
- One tanh ACT per PAIR of batches (sum/ha tiles hold 2 batches) to
  amortize the 352-cycle ACT overhead.
- Scores keep the v3 masked-wa DoubleRow trick (16 MMs/group into a
  [4,197] psum tile, batched softmax, no score transposes).
- visAtt keeps the v3 masked-pi-column trick (bf16 pi, N=512 MMs
  accumulating all batches into [16,1024] psum), one transpose pass at
  the end.
- DMA stream order: w_ho, w_hoe, cfe01, conv0, w_fr, w_fre, conv1,
  cfe23, conv23, w_h — each tensor arrives just before first use.
"""

import sys

if "/opt/trn_rl_repo" not in sys.path:
    sys.path.insert(0, "/opt/trn_rl_repo")

import numpy as np

import concourse.bass as bass
import concourse.tile as tile
from concourse import mybir
from concourse import bacc
from concourse.bass_utils import run_bass_kernel_spmd
from concourse.masks import make_identity

# ---------------------------------------------------------------------------

B, L, D = 128, 196, 1024
N_CORES = 8
S = B // N_CORES          # batches per core
CH = D // 128             # 128-wide chunks of D
LC = 98                   # conv l-chunk rows; lc0 holds l0-slot + l=1..98
LP = 208                  # ha l pitch: col0 = l0, cols 1..196 = conv l
G = 4                     # batches per softmax group
NG = S // G

F32 = mybir.dt.float32
BF16 = mybir.dt.bfloat16
F8 = mybir.dt.float8e4

ACTF = mybir.ActivationFunctionType
ALU = mybir.AluOpType
DR = mybir.MatmulPerfMode.DoubleRow

# NOTE: no GpSimd offload — concurrent GpSimd tensor ops slow co-running
# DVE ops ~2.5x via the shared SBUF port (measured: DVE adds 1.78us solo
# vs 4.4us next to a GpSimd add), a net throughput loss.


def build_nc() -> bass.Bass:
    nc = bacc.Bacc()

    def param(name, shape, dt, out=False):
        return nc.declare_dram_parameter(name, list(shape), dt, isOutput=out)

    xfr_d = param("xfr_T", (128, CH, S), BF16)
    xho_d = param("xho_T", (128, CH, S), BF16)
    w_d = {
        "ho": param("w_ho", (128, CH, D), BF16),
        "hoe": param("w_hoe", (128, CH, D), F8),
        "fr": param("w_fr", (128, CH, D), F8),
        "fre": param("w_fre", (128, CH, D), F8),
        "h": param("w_h", (128, CH, D), BF16),
    }
    bias_d = param("bias_row", (1, 5, D), BF16)
    waz_d = param("wa_z", (128, CH, G, G), F8)
    cfe_d = param("cfe8", (128, S, CH, L), F8)
    conv_d = param("conv8", (LC + 1, S, 2, D), F8)
    out_d = param("out", (S, D), F32, out=True)

    LI = {"fr": 0, "fre": 1, "ho": 2, "hoe": 3, "h": 4}

    with tile.TileContext(nc) as tc:
        with (
            tc.tile_pool(name="singles", bufs=1) as singles,
            tc.tile_pool(name="w16p", bufs=2) as w16p,
            tc.tile_pool(name="w8p", bufs=6) as w8p,
            tc.tile_pool(name="acts", bufs=1) as acts,
            tc.tile_pool(name="cfep", bufs=2) as cfep,
            tc.tile_pool(name="hap", bufs=3) as hap,
            tc.tile_pool(name="sump", bufs=3) as sump,
            tc.tile_pool(name="convp", bufs=NG) as convp,
            tc.tile_pool(name="smx", bufs=3) as smx,
            tc.tile_pool(name="ps_mm", bufs=2, space="PSUM") as ps_mm,
            tc.tile_pool(name="ps_sc", bufs=2, space="PSUM") as ps_sc,
            tc.tile_pool(name="ps_pi", bufs=2, space="PSUM") as ps_pi,
            tc.tile_pool(name="ps_va", bufs=2, space="PSUM") as ps_va,
        ):
            # --- small loads + weight/stream DMAs in need order ------------
            xho_t = singles.tile([128, CH, S], BF16)
            nc.sync.dma_start(out=xho_t, in_=xho_d[:, :, :])
            bias_t = singles.tile([1, 5, D], BF16)
            nc.sync.dma_start(out=bias_t, in_=bias_d[:, :, :])
            xfr_t = singles.tile([128, CH, S], BF16)
            nc.sync.dma_start(out=xfr_t, in_=xfr_d[:, :, :])
            waz_t = singles.tile([128, CH, G, G], F8)
            nc.sync.dma_start(out=waz_t, in_=waz_d[:, :, :, :])

            def wload(lname, wpool, wdt):
                w_c = []
                for kc in range(2):
                    t = wpool.tile(
                        [128, 4, D], wdt, tag=f"w_{wdt}",
                        name=f"w_{lname}_{kc}",
                    )
                    nc.sync.dma_start(
                        out=t, in_=w_d[lname][:, 4 * kc:4 * kc + 4, :]
                    )
                    w_c.append(t)
                return w_c

            def cfe_load(q):
                t = cfep.tile(
                    [128, G, CH, L], F8, tag="cfe", name=f"cfe_{q}"
                )
                nc.sync.dma_start(out=t, in_=cfe_d[:, G * q:G * q + G, :, :])
                return t

            def conv_load(q):
                t = convp.tile(
                    [LC + 1, G, 2, D], F8, tag="conv", name=f"conv_{q}"
                )
                nc.sync.dma_start(out=t, in_=conv_d[:, G * q:G * q + G, :, :])
                return t

            w_ho_c = wload("ho", w16p, BF16)
            w_hoe_c = wload("hoe", w8p, F8)
            cfe_q = [cfe_load(0), cfe_load(1)]
            conv_q = [conv_load(0)]
            w_fr_c = wload("fr", w8p, F8)
            w_fre_c = wload("fre", w8p, F8)
            conv_q.append(conv_load(1))
            cfe_q += [cfe_load(2), cfe_load(3)]
            conv_q += [conv_load(2), conv_load(3)]
            w_h_c = wload("h", w16p, BF16)

            ones_t = singles.tile([1, S], BF16)
            nc.vector.memset(ones_t, 1.0)
            id_bf = singles.tile([128, 128], BF16)
            make_identity(nc, id_bf)

            # --- dense layers (W stationary, one psum bank per layer) ------
            def dense(lname, rhs_sb, func, out_dt, w_c):
                ps = ps_mm.tile([128, CH, S], F32, tag="mm")
                li = LI[lname]
                for o in range(CH):
                    nc.tensor.matmul(
                        ps[:, o, :],
                        lhsT=bias_t[0:1, li, o * 128:(o + 1) * 128],
                        rhs=ones_t,
                        start=(o == 0), stop=False,
                        tile_position=(0, 0),
                    )
                for kc in range(2):
                    for k in range(4):
                        for o in range(CH):
                            last = kc == 1 and k == 3 and o == CH - 1
                            nc.tensor.matmul(
                                ps[:, o, :],
                                lhsT=w_c[kc][:, k, o * 128:(o + 1) * 128],
                                rhs=rhs_sb[:, 4 * kc + k, :],
                                start=False, stop=last,
                            )
                out_sb = acts.tile([128, CH, S], out_dt, tag=f"act_{lname}")
                nc.scalar.activation(
                    out=out_sb.rearrange("p c b -> p (c b)"),
                    in_=ps.rearrange("p c b -> p (c b)"),
                    func=func,
                )
                return out_sb

            ho_t = dense("ho", xho_t, ACTF.Tanh, BF16, w_ho_c)
            hoe_t = dense("hoe", ho_t, ACTF.Identity, BF16, w_hoe_c)

            # --- fr chain: fr, fre, row-0 inject, ha0 ----------------------
            fr_t = dense("fr", xfr_t, ACTF.Relu, BF16, w_fr_c)
            fre_t = dense("fre", fr_t, ACTF.Identity, BF16, w_fre_c)

            frn_ps = ps_pi.tile([16, CH, 128], BF16, tag="pi", name="frn")
            for c in range(CH):
                nc.tensor.transpose(frn_ps[:, c, :], fr_t[:, c, :], id_bf)
            fr_nat8 = singles.tile([16, CH, 128], F8)
            nc.scalar.activation(
                out=fr_nat8.rearrange("b c p -> b (c p)"),
                in_=frn_ps.rearrange("b c p -> b (c p)"),
                func=ACTF.Copy,
            )
            # row-0 injects ride the scalar HWDGE ring so they don't queue
            # behind the big streams on the sync ring
            for q in range(NG):
                nc.scalar.dma_start(
                    out=conv_q[q][0:1, :, 0, :],
                    in_=fr_nat8[G * q:G * q + G, :, :],
                )

            sum0 = acts.tile([128, CH, S], BF16, tag="sum0")
            nc.vector.tensor_add(sum0, fre_t, hoe_t)
            ha0 = acts.tile([128, CH, S], F8, tag="ha0")
            nc.scalar.activation(
                out=ha0.rearrange("p c b -> p (c b)"),
                in_=sum0.rearrange("p c b -> p (c b)"),
                func=ACTF.Tanh,
            )

            # --- batch loop with interleaved per-group softmax/visAtt ------
            sc_g = [
                ps_sc.tile([G, 1 + L], F32, tag="sc", name=f"sc_{g}")
                for g in range(NG)
            ]
            z_t = singles.tile([LC + 1, 2, S, S], BF16)
            nc.vector.memset(z_t, 0.0)
            va_ps = [
                ps_va.tile([S, 512], F32, tag="va", name=f"va_{h}")
                for h in range(2)
            ]

            ha_pair = None
            for b in range(S):
                q, j, jj = b // G, b % G, b % 2
                if jj == 0:
                    sum_pair = sump.tile([128, 2, CH, L], BF16, tag="sum")
                    ha_pair = hap.tile([128, 2, CH, LP], F8, tag="ha")
                nc.vector.tensor_tensor(
                    sum_pair[:, jj, :, :],
                    cfe_q[q][:, j, :, :],
                    hoe_t[:, :, b:b + 1].broadcast_to([128, CH, L]),
                    op=ALU.add,
                )
                nc.vector.tensor_copy(
                    ha_pair[:, jj, :, 0:1], ha0[:, :, b:b + 1]
                )
                if jj == 1:
                    nc.scalar.activation(
                        out=ha_pair[:, :, :, 1:1 + L],
                        in_=sum_pair,
                        func=ACTF.Tanh,
                    )
                    for bb in (b - 1, b):
                        jb = bb % G
                        for sp in range(4):
                            nc.tensor.matmul(
                                sc_g[q][:, :],
                                lhsT=waz_t[:, 2 * sp:2 * sp + 2, jb, :],
                                rhs=ha_pair[:, bb % 2,
                                            2 * sp:2 * sp + 2, 0:1 + L],
                                start=(jb == 0 and sp == 0),
                                stop=(jb == G - 1 and sp == 3),
                                perf_mode=DR,
                            )

                # group g's softmax/visAtt emits ONE PAIR after its last
                # score matmul: the next pair's two adds cover the
                # tanh->scores latency so the blocking tensor_reduce never
                # parks at the DVE queue head. g3 emits at b15 (no later
                # slot exists).
                g = None
                if jj == 1 and (b // 2) % 2 == 0 and b >= 4:
                    g = b // 4 - 1
                if b == S - 1:
                    g = NG - 1
                if g is not None:
                    # softmax for group g on psum rows [4, 197]
                    neg_mx = smx.tile([G, 1], F32, tag="negmx")
                    nc.vector.tensor_reduce(
                        out=neg_mx, in_=sc_g[g],
                        axis=mybir.AxisListType.X, op=ALU.max, negate=True,
                    )
                    exp_t = smx.tile([G, 1 + L], F32, tag="exp")
                    nc.scalar.activation(
                        out=exp_t, in_=sc_g[g],
                        func=ACTF.Exp, bias=neg_mx, scale=1.0,
                    )
                    ssum = smx.tile([G, 1], F32, tag="ssum")
                    nc.vector.tensor_reduce(
                        out=ssum, in_=exp_t,
                        axis=mybir.AxisListType.X, op=ALU.add,
                    )
                    rsum = smx.tile([G, 1], F32, tag="rsum")
                    nc.vector.reciprocal(rsum, ssum)
                    pi_sb = smx.tile([G, 1 + L], BF16, tag="pi")
                    nc.vector.tensor_scalar_mul(pi_sb, exp_t, rsum)

                    pi_ps = ps_pi.tile(
                        [LC + 1, 2, G], BF16, tag="pi", name=f"pi_{g}"
                    )
                    nc.tensor.transpose(
                        pi_ps[:, 0, :], pi_sb[:, 0:LC + 1],
                        id_bf[0:G, 0:G],
                    )
                    nc.tensor.transpose(
                        pi_ps[0:LC, 1, :], pi_sb[:, LC + 1:1 + L],
                        id_bf[0:G, 0:G],
                    )

                    for jv in range(G):
                        bv = G * g + jv
                        nc.vector.tensor_copy(
                            z_t[:, :, bv, bv:bv + 1], pi_ps[:, :, jv:jv + 1]
                        )
                        cq = conv_q[g]
                        for lc in range(2):
                            rows = LC + 1 if lc == 0 else LC
                            for h in range(2):
                                nc.tensor.matmul(
                                    va_ps[h][:, :],
                                    lhsT=z_t[0:rows, lc, bv, :],
                                    rhs=cq[0:rows, jv, lc,
                                           512 * h:512 * h + 512],
                                    start=(bv == 0 and lc == 0),
                                    stop=(bv == S - 1 and lc == 1),
                                )

            # --- attn = visAtt + ho (transpose back); h dense --------------
            va_sb = acts.tile([S, D], BF16, tag="va_sb")
            nc.scalar.activation(
                out=va_sb[:, 0:512], in_=va_ps[0], func=ACTF.Copy,
            )
            nc.vector.tensor_copy(va_sb[:, 512:1024], va_ps[1])
            attn_ps = ps_va.tile([128, CH, S], BF16, tag="va", name="attn")
            for c in range(CH):
                nc.tensor.transpose(
                    attn_ps[:, c, :], va_sb[:, c * 128:(c + 1) * 128],
                    id_bf[0:S, 0:S],
                )
            attn = acts.tile([128, CH, S], BF16, tag="attn")
            nc.vector.tensor_add(attn, attn_ps, ho_t)
            # h dense FLIPPED: attn k-chunks are the 16-column stationary
            # (13ns LDW vs ~104ns for a 128-col weight tile), W_h streams
            # at N=512 — 18 matmuls instead of 72, h lands in row layout
            # and the host-side transpose disappears.
            h_ps = [
                ps_mm.tile([S, 512], F32, tag="mm", name=f"h_ps_{hh}")
                for hh in range(2)
            ]
            for hh in range(2):
                nc.tensor.matmul(
                    h_ps[hh][:, :],
                    lhsT=ones_t,
                    rhs=bias_t[0:1, 4, 512 * hh:512 * hh + 512],
                    start=True, stop=False,
                    tile_position=(0, 0),
                )
            for kc in range(2):
                for k in range(4):
                    for hh in range(2):
                        nc.tensor.matmul(
                            h_ps[hh][:, :],
                            lhsT=attn[:, 4 * kc + k, :],
                            rhs=w_h_c[kc][:, k, 512 * hh:512 * hh + 512],
                            start=False, stop=(kc == 1 and k == 3),
                        )
            h_sb = acts.tile([S, D], F32, tag="act_h")
            for hh in range(2):
                nc.scalar.activation(
                    out=h_sb[:, 512 * hh:512 * hh + 512],
                    in_=h_ps[hh], func=ACTF.Tanh,
                )
            nc.sync.dma_start(out=out_d[:, :], in_=h_sb)

    return nc


# ---------------------------------------------------------------------------

_NC_CACHE = {}


def _get_nc():
    if "nc" not in _NC_CACHE:
        nc = build_nc()
        nc.compile()
        _NC_CACHE["nc"] = nc
    return _NC_CACHE["nc"]


F8NP = mybir.dt.np(F8)
BFNP = mybir.dt.np(BF16)


def make_in_maps(inputs):
    def wpack(w, dt):
        # [128, CH, D]: w[p, k, o] = W[o, k*128+p]
        return np.ascontiguousarray(
            w.T.reshape(CH, 128, D).transpose(1, 0, 2).astype(dt)
        )

    shared = {
        "w_ho": wpack(np.asarray(inputs["W_ho"]), BFNP),
        "w_h": wpack(np.asarray(inputs["W_h"]), BFNP),
        "w_hoe": wpack(np.asarray(inputs["W_hoe"]), F8NP),
        "w_fr": wpack(np.asarray(inputs["W_fr"]), F8NP),
        "w_fre": wpack(np.asarray(inputs["W_fre"]), F8NP),
    }
    bias_row = np.stack(
        [np.asarray(inputs[f"b_{n}"]) for n in ("fr", "fre", "ho", "hoe", "h")]
    )  # [5, D]
    shared["bias_row"] = np.ascontiguousarray(bias_row[None].astype(BFNP))
    # wa_z[p, c, j, col] = wa[c*128+p] if col == j else 0
    wa_col = np.asarray(inputs["W_a"]).reshape(CH, 128).T.astype(F8NP)
    wa_z = np.zeros((128, CH, G, G), F8NP)
    for j in range(G):
        wa_z[:, :, j, j] = wa_col
    shared["wa_z"] = wa_z

    cfe_all = np.asarray(inputs["conv_feat_embed"])
    conv_all = np.asarray(inputs["conv_feat"])

    in_maps = []
    for i in range(N_CORES):
        sl = slice(i * S, (i + 1) * S)
        m = dict(shared)

        def xpack(x):
            # [128, CH, S]: x[p, k, b] = v[b, k*128+p]
            return np.ascontiguousarray(
                x.T.reshape(CH, 128, S).transpose(1, 0, 2).astype(BFNP)
            )

        m["xfr_T"] = xpack(np.asarray(inputs["fake_region"])[sl])
        m["xho_T"] = xpack(np.asarray(inputs["h_out"])[sl])

        # cfe8[p, b, s, l] = cfe[b, l, s*128+p]
        m["cfe8"] = np.ascontiguousarray(
            cfe_all[sl].transpose(2, 0, 1).reshape(CH, 128, S, L)
            .transpose(1, 2, 0, 3).astype(F8NP)
        )

        # conv8 lc0: row0 = l0 slot (runtime fr inject), rows 1..98 =
        # conv l 0..97; lc1: rows 0..97 = conv l 98..195
        conv8 = np.zeros((LC + 1, S, 2, D), F8NP)
        cs = conv_all[sl].astype(F8NP)          # [S, L, D]
        conv8[1:LC + 1, :, 0, :] = cs[:, 0:LC, :].transpose(1, 0, 2)
        conv8[0:LC, :, 1, :] = cs[:, LC:L, :].transpose(1, 0, 2)
        m["conv8"] = conv8
        in_maps.append(m)
    return in_maps


def run(inputs, trace=False, trace_kwargs=None):
    nc = _get_nc()
    in_maps = make_in_maps(inputs)
    res = run_bass_kernel_spmd(
        nc, in_maps, core_ids=list(range(N_CORES)), trace=trace,
        **(trace_kwargs or {}),
    )
    shards = [res.results[i]["out"] for i in range(N_CORES)]
    # out is already [S, D] rows per core
    h = np.concatenate(shards, axis=0).astype(np.float32)
    return h, res


def kernel(**inputs) -> np.ndarray:
    h, _ = run(inputs, trace=False)
    return h


if __name__ == "__main__":
    nc = build_nc()
    print(f"built ok: {len(nc.inst_map)} instructions")


# revision 27
# speedup vs baseline: 1.0027x; 1.0027x over previous
"""AdaAtt attention block on 8 TRN2 NeuronCores — v4.

Data-parallel over batch (16/core), weights replicated.

v3 trace (106us) postmortem: the batch loop and the softmax/visAtt loop
were separate, so on the strict-FIFO ACT/DVE queues every softmax op
queued behind all 16 tanh instructions (g0's reciprocal ran 33us after
its inputs were ready) and the whole visAtt+tail serialized after the
batch phase. tensor_scalar_add also ran in 1x mode (the scalar operand
is forced f32, which disqualifies the packed modes), so the fp8->bf16
cast-DMA bought nothing and its 6.4MB of SBUF writes slowed the weight
stream.

v4:
- Softmax + pi transpose + masked visAtt MMs interleaved into the batch
  loop right after each group's last score matmul.
- Adds are plain tensor_tensor (fp8 cfe + stride-0 hoe broadcast, 1x) at
  ~1.7us/batch, with 4 of 16 batches on the otherwise-idle GpSimd; cfe
  loads move back to the sync HWDGE queue (no cast, no SWDGE).
- One tanh ACT per PAIR of batches (sum/ha tiles hold 2 batches) to
  amortize the 352-cycle ACT overhead.
- Scores keep the v3 masked-wa DoubleRow trick (16 MMs/group into a
  [4,197] psum tile, batched softmax, no score transposes).
- visAtt keeps the v3 masked-pi-column trick (bf16 pi, N=512 MMs
  accumulating all batches into [16,1024] psum), one transpose pass at
  the end.
- DMA stream order: w_ho, w_hoe, cfe01, conv0, w_fr, w_fre, conv1,
  cfe23, conv23, w_h — each tensor arrives just before first use.
"""

import sys

if "/opt/trn_rl_repo" not in sys.path:
    sys.path.insert(0, "/opt/trn_rl_repo")

import numpy as np

import concourse.bass as bass
import concourse.tile as tile
from concourse import mybir
from concourse import bacc
from concourse.bass_utils import run_bass_kernel_spmd
from concourse.masks import make_identity

# ---------------------------------------------------------------------------

B, L, D = 128, 196, 1024
N_CORES = 8
S = B // N_CORES          # batches per core
CH = D // 128             # 128-wide chunks of D
LC = 98                   # conv l-chunk rows; lc0 holds l0-slot + l=1..98
LP = 208                  # ha l pitch: col0 = l0, cols 1..196 = conv l
G = 4                     # batches per softmax group
NG = S // G

F32 = mybir.dt.float32
BF16 = mybir.dt.bfloat16
F8 = mybir.dt.float8e4

ACTF = mybir.ActivationFunctionType
ALU = mybir.AluOpType
DR = mybir.MatmulPerfMode.DoubleRow

# NOTE: no GpSimd offload — concurrent GpSimd tensor ops slow co-running
# DVE ops ~2.5x via the shared SBUF port (measured: DVE adds 1.78us solo
# vs 4.4us next to a GpSimd add), a net throughput loss.


def build_nc() -> bass.Bass:
    nc = bacc.Bacc()

    def param(name, shape, dt, out=False):
        return nc.declare_dram_parameter(name, list(shape), dt, isOutput=out)

    xfr_d = param("xfr_T", (128, CH, S), BF16)
    xho_d = param("xho_T", (128, CH, S), BF16)
    w_d = {
        "ho": param("w_ho", (128, CH, D), BF16),
        "hoe": param("w_hoe", (128, CH, D), F8),
        "fr": param("w_fr", (128, CH, D), F8),
        "fre": param("w_fre", (128, CH, D), F8),
        "h": param("w_h", (128, CH, D), BF16),
    }
    bias_d = param("bias_row", (1, 5, D), BF16)
    waz_d = param("wa_z", (128, CH, G, G), F8)
    cfe_d = param("cfe8", (128, S, CH, L), F8)
    conv_d = param("conv8", (LC + 1, S, 2, D), F8)
    out_d = param("out", (128, CH, S), F32, out=True)

    LI = {"fr": 0, "fre": 1, "ho": 2, "hoe": 3, "h": 4}

    with tile.TileContext(nc) as tc:
        with (
            tc.tile_pool(name="singles", bufs=1) as singles,
            tc.tile_pool(name="w16p", bufs=2) as w16p,
            tc.tile_pool(name="w8p", bufs=6) as w8p,
            tc.tile_pool(name="acts", bufs=1) as acts,
            tc.tile_pool(name="cfep", bufs=2) as cfep,
            tc.tile_pool(name="hap", bufs=3) as hap,
            tc.tile_pool(name="sump", bufs=3) as sump,
            tc.tile_pool(name="convp", bufs=NG) as convp,
            tc.tile_pool(name="smx", bufs=3) as smx,
            tc.tile_pool(name="ps_mm", bufs=2, space="PSUM") as ps_mm,
            tc.tile_pool(name="ps_sc", bufs=2, space="PSUM") as ps_sc,
            tc.tile_pool(name="ps_pi", bufs=2, space="PSUM") as ps_pi,
            tc.tile_pool(name="ps_va", bufs=2, space="PSUM") as ps_va,
        ):
            # --- small loads + weight/stream DMAs in need order ------------
            xho_t = singles.tile([128, CH, S], BF16)
            nc.sync.dma_start(out=xho_t, in_=xho_d[:, :, :])
            bias_t = singles.tile([1, 5, D], BF16)
            nc.sync.dma_start(out=bias_t, in_=bias_d[:, :, :])
            xfr_t = singles.tile([128, CH, S], BF16)
            nc.sync.dma_start(out=xfr_t, in_=xfr_d[:, :, :])
            waz_t = singles.tile([128, CH, G, G], F8)
            nc.sync.dma_start(out=waz_t, in_=waz_d[:, :, :, :])

            def wload(lname, wpool, wdt):
                w_c = []
                for kc in range(2):
                    t = wpool.tile(
                        [128, 4, D], wdt, tag=f"w_{wdt}",
                        name=f"w_{lname}_{kc}",
                    )
                    nc.sync.dma_start(
                        out=t, in_=w_d[lname][:, 4 * kc:4 * kc + 4, :]
                    )
                    w_c.append(t)
                return w_c

            def cfe_load(q):
                t = cfep.tile(
                    [128, G, CH, L], F8, tag="cfe", name=f"cfe_{q}"
                )
                nc.sync.dma_start(out=t, in_=cfe_d[:, G * q:G * q + G, :, :])
                return t

            def conv_load(q):
                t = convp.tile(
                    [LC + 1, G, 2, D], F8, tag="conv", name=f"conv_{q}"
                )
                nc.sync.dma_start(out=t, in_=conv_d[:, G * q:G * q + G, :, :])
                return t

            w_ho_c = wload("ho", w16p, BF16)
            w_hoe_c = wload("hoe", w8p, F8)
            cfe_q = [cfe_load(0), cfe_load(1)]
            conv_q = [conv_load(0)]
            w_fr_c = wload("fr", w8p, F8)
            w_fre_c = wload("fre", w8p, F8)
            conv_q.append(conv_load(1))
            cfe_q += [cfe_load(2), cfe_load(3)]
            conv_q += [conv_load(2), conv_load(3)]
            w_h_c = wload("h", w16p, BF16)

            ones_t = singles.tile([1, S], BF16)
            nc.vector.memset(ones_t, 1.0)
            id_bf = singles.tile([128, 128], BF16)
            make_identity(nc, id_bf)

            # --- dense layers (W stationary, one psum bank per layer) ------
            def dense(lname, rhs_sb, func, out_dt, w_c):
                ps = ps_mm.tile([128, CH, S], F32, tag="mm")
                li = LI[lname]
                for o in range(CH):
                    nc.tensor.matmul(
                        ps[:, o, :],
                        lhsT=bias_t[0:1, li, o * 128:(o + 1) * 128],
                        rhs=ones_t,
                        start=(o == 0), stop=False,
                        tile_position=(0, 0),
                    )
                for kc in range(2):
                    for k in range(4):
                        for o in range(CH):
                            last = kc == 1 and k == 3 and o == CH - 1
                            nc.tensor.matmul(
                                ps[:, o, :],
                                lhsT=w_c[kc][:, k, o * 128:(o + 1) * 128],
                                rhs=rhs_sb[:, 4 * kc + k, :],
                                start=False, stop=last,
                            )
                out_sb = acts.tile([128, CH, S], out_dt, tag=f"act_{lname}")
                nc.scalar.activation(
                    out=out_sb.rearrange("p c b -> p (c b)"),
                    in_=ps.rearrange("p c b -> p (c b)"),
                    func=func,
                )
                return out_sb

            ho_t = dense("ho", xho_t, ACTF.Tanh, BF16, w_ho_c)
            hoe_t = dense("hoe", ho_t, ACTF.Identity, BF16, w_hoe_c)

            # --- fr chain: fr, fre, row-0 inject, ha0 ----------------------
            fr_t = dense("fr", xfr_t, ACTF.Relu, BF16, w_fr_c)
            fre_t = dense("fre", fr_t, ACTF.Identity, BF16, w_fre_c)

            frn_ps = ps_pi.tile([16, CH, 128], BF16, tag="pi", name="frn")
            for c in range(CH):
                nc.tensor.transpose(frn_ps[:, c, :], fr_t[:, c, :], id_bf)
            fr_nat8 = singles.tile([16, CH, 128], F8)
            nc.scalar.activation(
                out=fr_nat8.rearrange("b c p -> b (c p)"),
                in_=frn_ps.rearrange("b c p -> b (c p)"),
                func=ACTF.Copy,
            )
            # row-0 injects ride the scalar HWDGE ring so they don't queue
            # behind the big streams on the sync ring
            for q in range(NG):
                nc.scalar.dma_start(
                    out=conv_q[q][0:1, :, 0, :],
                    in_=fr_nat8[G * q:G * q + G, :, :],
                )

            sum0 = acts.tile([128, CH, S], BF16, tag="sum0")
            nc.vector.tensor_add(sum0, fre_t, hoe_t)
            ha0 = acts.tile([128, CH, S], F8, tag="ha0")
            nc.scalar.activation(
                out=ha0.rearrange("p c b -> p (c b)"),
                in_=sum0.rearrange("p c b -> p (c b)"),
                func=ACTF.Tanh,
            )

            # --- batch loop with interleaved per-group softmax/visAtt ------
            sc_g = [
                ps_sc.tile([G, 1 + L], F32, tag="sc", name=f"sc_{g}")
                for g in range(NG)
            ]
            # z flat last dim [b*S + col]; pi_b lives at diagonal b*(S+1)
            z_t = singles.tile([LC + 1, 2, S * S], BF16)
            nc.vector.memset(z_t, 0.0)
            va_ps = [
                ps_va.tile([S, 512], F32, tag="va", name=f"va_{h}")
                for h in range(2)
            ]

            ha_pair = None
            for b in range(S):
                q, j, jj = b // G, b % G, b % 2
                if jj == 0:
                    sum_pair = sump.tile([128, 2, CH, L], BF16, tag="sum")
                    ha_pair = hap.tile([128, 2, CH, LP], F8, tag="ha")
                    # both batches' l0 slots in one permuted-AP copy
                    nc.vector.tensor_copy(
                        ha_pair[:, :, :, 0:1],
                        ha0[:, :, b:b + 2].rearrange("p c b2 -> p b2 c"),
                    )
                nc.vector.tensor_tensor(
                    sum_pair[:, jj, :, :],
                    cfe_q[q][:, j, :, :],
                    hoe_t[:, :, b:b + 1].broadcast_to([128, CH, L]),
                    op=ALU.add,
                )
                if jj == 1:
                    nc.scalar.activation(
                        out=ha_pair[:, :, :, 1:1 + L],
                        in_=sum_pair,
                        func=ACTF.Tanh,
                    )
                    for bb in (b - 1, b):
                        jb = bb % G
                        for sp in range(4):
                            nc.tensor.matmul(
                                sc_g[q][:, :],
                                lhsT=waz_t[:, 2 * sp:2 * sp + 2, jb, :],
                                rhs=ha_pair[:, bb % 2,
                                            2 * sp:2 * sp + 2, 0:1 + L],
                                start=(jb == 0 and sp == 0),
                                stop=(jb == G - 1 and sp == 3),
                                perf_mode=DR,
                            )

                # group g's softmax/visAtt emits ONE PAIR after its last
                # score matmul: the next pair's two adds cover the
                # tanh->scores latency so the blocking tensor_reduce never
                # parks at the DVE queue head. g3 emits at b15 (no later
                # slot exists).
                g = None
                if jj == 1 and (b // 2) % 2 == 0 and b >= 4:
                    g = b // 4 - 1
                if b == S - 1:
                    g = NG - 1
                if g is not None:
                    # softmax for group g on psum rows [4, 197]
                    neg_mx = smx.tile([G, 1], F32, tag="negmx")
                    nc.vector.tensor_reduce(
                        out=neg_mx, in_=sc_g[g],
                        axis=mybir.AxisListType.X, op=ALU.max, negate=True,
                    )
                    exp_t = smx.tile([G, 1 + L], F32, tag="exp")
                    nc.scalar.activation(
                        out=exp_t, in_=sc_g[g],
                        func=ACTF.Exp, bias=neg_mx, scale=1.0,
                    )
                    ssum = smx.tile([G, 1], F32, tag="ssum")
                    nc.vector.tensor_reduce(
                        out=ssum, in_=exp_t,
                        axis=mybir.AxisListType.X, op=ALU.add,
                    )
                    rsum = smx.tile([G, 1], F32, tag="rsum")
                    nc.vector.reciprocal(rsum, ssum)
                    pi_sb = smx.tile([G, 1 + L], BF16, tag="pi")
                    nc.vector.tensor_scalar_mul(pi_sb, exp_t, rsum)

                    pi_ps = ps_pi.tile(
                        [LC + 1, 2, G], BF16, tag="pi", name=f"pi_{g}"
                    )
                    nc.tensor.transpose(
                        pi_ps[:, 0, :], pi_sb[:, 0:LC + 1],
                        id_bf[0:G, 0:G],
                    )
                    nc.tensor.transpose(
                        pi_ps[0:LC, 1, :], pi_sb[:, LC + 1:1 + L],
                        id_bf[0:G, 0:G],
                    )

                    # one strided copy fills the group's 4 diagonal slots
                    d0 = (S + 1) * G * g
                    nc.vector.tensor_copy(
                        z_t[:, :, d0:d0 + 3 * (S + 1) + 1:S + 1],
                        pi_ps[:, :, 0:G],
                    )
                    for jv in range(G):
                        bv = G * g + jv
                        cq = conv_q[g]
                        for lc in range(2):
                            rows = LC + 1 if lc == 0 else LC
                            for h in range(2):
                                nc.tensor.matmul(
                                    va_ps[h][:, :],
                                    lhsT=z_t[0:rows, lc,
                                             S * bv:S * bv + S],
                                    rhs=cq[0:rows, jv, lc,
                                           512 * h:512 * h + 512],
                                    start=(bv == 0 and lc == 0),
                                    stop=(bv == S - 1 and lc == 1),
                                )

            # --- attn = visAtt + ho (transpose back); h dense --------------
            va_sb = acts.tile([S, D], BF16, tag="va_sb")
            nc.scalar.activation(
                out=va_sb[:, 0:512], in_=va_ps[0], func=ACTF.Copy,
            )
            nc.vector.tensor_copy(va_sb[:, 512:1024], va_ps[1])
            attn_ps = ps_va.tile([128, CH, S], BF16, tag="va", name="attn")
            for c in range(CH):
                nc.tensor.transpose(
                    attn_ps[:, c, :], va_sb[:, c * 128:(c + 1) * 128],
                    id_bf[0:S, 0:S],
                )
            attn = acts.tile([128, CH, S], BF16, tag="attn")
            nc.vector.tensor_add(attn, attn_ps, ho_t)
            h_sb = dense("h", attn, ACTF.Tanh, F32, w_h_c)
            nc.sync.dma_start(out=out_d[:, :, :], in_=h_sb)

    return nc


# ---------------------------------------------------------------------------

_NC_CACHE = {}


def _get_nc():
    if "nc" not in _NC_CACHE:
        nc = build_nc()
        nc.compile()
        _NC_CACHE["nc"] = nc
    return _NC_CACHE["nc"]


F8NP = mybir.dt.np(F8)
BFNP = mybir.dt.np(BF16)


def make_in_maps(inputs):
    def wpack(w, dt):
        # [128, CH, D]: w[p, k, o] = W[o, k*128+p]
        return np.ascontiguousarray(
            w.T.reshape(CH, 128, D).transpose(1, 0, 2).astype(dt)
        )

    shared = {
        "w_ho": wpack(np.asarray(inputs["W_ho"]), BFNP),
        "w_h": wpack(np.asarray(inputs["W_h"]), BFNP),
        "w_hoe": wpack(np.asarray(inputs["W_hoe"]), F8NP),
        "w_fr": wpack(np.asarray(inputs["W_fr"]), F8NP),
        "w_fre": wpack(np.asarray(inputs["W_fre"]), F8NP),
    }
    bias_row = np.stack(
        [np.asarray(inputs[f"b_{n}"]) for n in ("fr", "fre", "ho", "hoe", "h")]
    )  # [5, D]
    shared["bias_row"] = np.ascontiguousarray(bias_row[None].astype(BFNP))
    # wa_z[p, c, j, col] = wa[c*128+p] if col == j else 0
    wa_col = np.asarray(inputs["W_a"]).reshape(CH, 128).T.astype(F8NP)
    wa_z = np.zeros((128, CH, G, G), F8NP)
    for j in range(G):
        wa_z[:, :, j, j] = wa_col
    shared["wa_z"] = wa_z

    cfe_all = np.asarray(inputs["conv_feat_embed"])
    conv_all = np.asarray(inputs["conv_feat"])

    in_maps = []
    for i in range(N_CORES):
        sl = slice(i * S, (i + 1) * S)
        m = dict(shared)

        def xpack(x):
            # [128, CH, S]: x[p, k, b] = v[b, k*128+p]
            return np.ascontiguousarray(
                x.T.reshape(CH, 128, S).transpose(1, 0, 2).astype(BFNP)
            )

        m["xfr_T"] = xpack(np.asarray(inputs["fake_region"])[sl])
        m["xho_T"] = xpack(np.asarray(inputs["h_out"])[sl])

        # cfe8[p, b, s, l] = cfe[b, l, s*128+p]
        m["cfe8"] = np.ascontiguousarray(
            cfe_all[sl].transpose(2, 0, 1).reshape(CH, 128, S, L)
            .transpose(1, 2, 0, 3).astype(F8NP)
        )

        # conv8 lc0: row0 = l0 slot (runtime fr inject), rows 1..98 =
        # conv l 0..97; lc1: rows 0..97 = conv l 98..195
        conv8 = np.zeros((LC + 1, S, 2, D), F8NP)
        cs = conv_all[sl].astype(F8NP)          # [S, L, D]
        conv8[1:LC + 1, :, 0, :] = cs[:, 0:LC, :].transpose(1, 0, 2)
        conv8[0:LC, :, 1, :] = cs[:, LC:L, :].transpose(1, 0, 2)
        m["conv8"] = conv8
        in_maps.append(m)
    return in_maps


def run(inputs, trace=False, trace_kwargs=None):
    nc = _get_nc()
    in_maps = make_in_maps(inputs)
    res = run_bass_kernel_spmd(
        nc, in_maps, core_ids=list(range(N_CORES)), trace=trace,
        **(trace_kwargs or {}),
    )
    shards = [res.results[i]["out"] for i in range(N_CORES)]
    # out[p, c, b] = h[b, c*128+p]
    h = np.concatenate(
        [s.transpose(2, 1, 0).reshape(S, D) for s in shards], axis=0
    ).astype(np.float32)
    return h, res


def kernel(**inputs) -> np.ndarray:
    h, _ = run(inputs, trace=False)
    return h


if __name__ == "__main__":
    nc = build_nc()
    print(f"built ok: {len(nc.inst_map)} instructions")


# revision 30
# speedup vs baseline: 1.0141x; 1.0114x over previous
"""AdaAtt attention block on 8 TRN2 NeuronCores — v4.

Data-parallel over batch (16/core), weights replicated.

v3 trace (106us) postmortem: the batch loop and the softmax/visAtt loop
were separate, so on the strict-FIFO ACT/DVE queues every softmax op
queued behind all 16 tanh instructions (g0's reciprocal ran 33us after
its inputs were ready) and the whole visAtt+tail serialized after the
batch phase. tensor_scalar_add also ran in 1x mode (the scalar operand
is forced f32, which disqualifies the packed modes), so the fp8->bf16
cast-DMA bought nothing and its 6.4MB of SBUF writes slowed the weight
stream.

v4:
- Softmax + pi transpose + masked visAtt MMs interleaved into the batch
  loop right after each group's last score matmul.
- Adds are plain tensor_tensor (fp8 cfe + stride-0 hoe broadcast, 1x) at
  ~1.7us/batch, with 4 of 16 batches on the otherwise-idle GpSimd; cfe
  loads move back to the sync HWDGE queue (no cast, no SWDGE).
- One tanh ACT per PAIR of batches (sum/ha tiles hold 2 batches) to
  amortize the 352-cycle ACT overhead.
- Scores keep the v3 masked-wa DoubleRow trick (16 MMs/group into a
  [4,197] psum tile, batched softmax, no score transposes).
- visAtt keeps the v3 masked-pi-column trick (bf16 pi, N=512 MMs
  accumulating all batches into [16,1024] psum), one transpose pass at
  the end.
- DMA stream order: w_ho, w_hoe, cfe01, conv0, w_fr, w_fre, conv1,
  cfe23, conv23, w_h — each tensor arrives just before first use.
"""

import sys

if "/opt/trn_rl_repo" not in sys.path:
    sys.path.insert(0, "/opt/trn_rl_repo")

import numpy as np

import concourse.bass as bass
import concourse.tile as tile
from concourse import mybir
from concourse import bacc
from concourse.bass_utils import run_bass_kernel_spmd
from concourse.masks import make_identity

# ---------------------------------------------------------------------------

B, L, D = 128, 196, 1024
N_CORES = 8
S = B // N_CORES          # batches per core
CH = D // 128             # 128-wide chunks of D
LC = 98                   # conv l-chunk rows; lc0 holds l0-slot + l=1..98
LP = 208                  # ha l pitch: col0 = l0, cols 1..196 = conv l
G = 4                     # batches per softmax group
NG = S // G

F32 = mybir.dt.float32
BF16 = mybir.dt.bfloat16
F8 = mybir.dt.float8e4

ACTF = mybir.ActivationFunctionType
ALU = mybir.AluOpType
DR = mybir.MatmulPerfMode.DoubleRow

# NOTE: no GpSimd offload — concurrent GpSimd tensor ops slow co-running
# DVE ops ~2.5x via the shared SBUF port (measured: DVE adds 1.78us solo
# vs 4.4us next to a GpSimd add), a net throughput loss.


def build_nc() -> bass.Bass:
    nc = bacc.Bacc()

    def param(name, shape, dt, out=False):
        return nc.declare_dram_parameter(name, list(shape), dt, isOutput=out)

    xfr_d = param("xfr_T", (128, CH, S), BF16)
    xho_d = param("xho_T", (128, CH, S), BF16)
    w_d = {
        "ho": param("w_ho", (128, CH, D), BF16),
        "hoe": param("w_hoe", (128, CH, D), F8),
        "fr": param("w_fr", (128, CH, D), F8),
        "fre": param("w_fre", (128, CH, D), F8),
        "h": param("w_h", (128, CH, D), BF16),
    }
    bias_d = param("bias_row", (1, 5, D), BF16)
    waz_d = param("wa_z", (128, CH, G, G), F8)
    cfe_d = param("cfe8", (128, S, CH, L), F8)
    conv_d = param("conv8", (LC + 1, S, 2, D), F8)
    out_d = param("out", (128, CH, S), F32, out=True)

    LI = {"fr": 0, "fre": 1, "ho": 2, "hoe": 3, "h": 4}

    with tile.TileContext(nc) as tc:
        with (
            tc.tile_pool(name="singles", bufs=1) as singles,
            tc.tile_pool(name="w16p", bufs=2) as w16p,
            tc.tile_pool(name="w8p", bufs=6) as w8p,
            tc.tile_pool(name="acts", bufs=1) as acts,
            tc.tile_pool(name="cfep", bufs=2) as cfep,
            tc.tile_pool(name="hap", bufs=3) as hap,
            tc.tile_pool(name="sump", bufs=3) as sump,
            tc.tile_pool(name="convp", bufs=NG) as convp,
            tc.tile_pool(name="smx", bufs=3) as smx,
            tc.tile_pool(name="ps_mm", bufs=2, space="PSUM") as ps_mm,
            tc.tile_pool(name="ps_sc", bufs=2, space="PSUM") as ps_sc,
            tc.tile_pool(name="ps_pi", bufs=2, space="PSUM") as ps_pi,
            tc.tile_pool(name="ps_va", bufs=2, space="PSUM") as ps_va,
        ):
            # --- small loads + weight/stream DMAs in need order ------------
            xho_t = singles.tile([128, CH, S], BF16)
            nc.sync.dma_start(out=xho_t, in_=xho_d[:, :, :])
            bias_t = singles.tile([1, 5, D], BF16)
            nc.sync.dma_start(out=bias_t, in_=bias_d[:, :, :])
            xfr_t = singles.tile([128, CH, S], BF16)
            nc.sync.dma_start(out=xfr_t, in_=xfr_d[:, :, :])
            waz_t = singles.tile([128, CH, G, G], F8)
            nc.sync.dma_start(out=waz_t, in_=waz_d[:, :, :, :])

            def wload(lname, wpool, wdt):
                w_c = []
                for kc in range(2):
                    t = wpool.tile(
                        [128, 4, D], wdt, tag=f"w_{wdt}",
                        name=f"w_{lname}_{kc}",
                    )
                    nc.sync.dma_start(
                        out=t, in_=w_d[lname][:, 4 * kc:4 * kc + 4, :]
                    )
                    w_c.append(t)
                return w_c

            def cfe_load(q):
                t = cfep.tile(
                    [128, G, CH, L], F8, tag="cfe", name=f"cfe_{q}"
                )
                nc.sync.dma_start(out=t, in_=cfe_d[:, G * q:G * q + G, :, :])
                return t

            def conv_load(q):
                t = convp.tile(
                    [LC + 1, G, 2, D], F8, tag="conv", name=f"conv_{q}"
                )
                nc.sync.dma_start(out=t, in_=conv_d[:, G * q:G * q + G, :, :])
                return t

            w_ho_c = wload("ho", w16p, BF16)
            w_hoe_c = wload("hoe", w8p, F8)
            cfe_q = [cfe_load(0), cfe_load(1)]
            conv_q = [conv_load(0)]
            w_fr_c = wload("fr", w8p, F8)
            w_fre_c = wload("fre", w8p, F8)
            conv_q.append(conv_load(1))
            cfe_q += [cfe_load(2), cfe_load(3)]
            conv_q += [conv_load(2), conv_load(3)]
            w_h_c = wload("h", w16p, BF16)

            ones_t = singles.tile([1, S], BF16)
            nc.vector.memset(ones_t, 1.0)
            id_bf = singles.tile([128, 128], BF16)
            make_identity(nc, id_bf)

            # --- dense layers (W stationary, one psum bank per layer) ------
            def dense(lname, rhs_sb, func, out_dt, w_c):
                ps = ps_mm.tile([128, CH, S], F32, tag="mm")
                li = LI[lname]
                for o in range(CH):
                    nc.tensor.matmul(
                        ps[:, o, :],
                        lhsT=bias_t[0:1, li, o * 128:(o + 1) * 128],
                        rhs=ones_t,
                        start=(o == 0), stop=False,
                        tile_position=(0, 0),
                    )
                for kc in range(2):
                    for k in range(4):
                        for o in range(CH):
                            last = kc == 1 and k == 3 and o == CH - 1
                            nc.tensor.matmul(
                                ps[:, o, :],
                                lhsT=w_c[kc][:, k, o * 128:(o + 1) * 128],
                                rhs=rhs_sb[:, 4 * kc + k, :],
                                start=False, stop=last,
                            )
                out_sb = acts.tile([128, CH, S], out_dt, tag=f"act_{lname}")
                nc.scalar.activation(
                    out=out_sb.rearrange("p c b -> p (c b)"),
                    in_=ps.rearrange("p c b -> p (c b)"),
                    func=func,
                )
                return out_sb

            ho_t = dense("ho", xho_t, ACTF.Tanh, BF16, w_ho_c)
            hoe_t = dense("hoe", ho_t, ACTF.Identity, BF16, w_hoe_c)

            # --- fr chain: fr, fre, row-0 inject, ha0 ----------------------
            fr_t = dense("fr", xfr_t, ACTF.Relu, BF16, w_fr_c)
            fre_t = dense("fre", fr_t, ACTF.Identity, BF16, w_fre_c)

            frn_ps = ps_pi.tile([16, CH, 128], BF16, tag="pi", name="frn")
            for c in range(CH):
                nc.tensor.transpose(frn_ps[:, c, :], fr_t[:, c, :], id_bf)
            fr_nat8 = singles.tile([16, CH, 128], F8)
            nc.scalar.activation(
                out=fr_nat8.rearrange("b c p -> b (c p)"),
                in_=frn_ps.rearrange("b c p -> b (c p)"),
                func=ACTF.Copy,
            )
            # row-0 injects ride the scalar HWDGE ring so they don't queue
            # behind the big streams on the sync ring
            for q in range(NG):
                nc.scalar.dma_start(
                    out=conv_q[q][0:1, :, 0, :],
                    in_=fr_nat8[G * q:G * q + G, :, :],
                )

            sum0 = acts.tile([128, CH, S], BF16, tag="sum0")
            nc.vector.tensor_add(sum0, fre_t, hoe_t)
            ha0 = acts.tile([128, CH, S], F8, tag="ha0")
            nc.scalar.activation(
                out=ha0.rearrange("p c b -> p (c b)"),
                in_=sum0.rearrange("p c b -> p (c b)"),
                func=ACTF.Tanh,
            )

            # --- batch loop with interleaved per-group softmax/visAtt ------
            sc_g = [
                ps_sc.tile([G, 1 + L], F32, tag="sc", name=f"sc_{g}")
                for g in range(NG)
            ]
            z_t = singles.tile([LC + 1, 2, S, S], BF16)
            nc.vector.memset(z_t, 0.0)
            va_ps = [
                ps_va.tile([S, 512], F32, tag="va", name=f"va_{h}")
                for h in range(2)
            ]

            pi_of = {}

            def group_chain(g):
                # softmax for group g on psum rows [4, 197]
                neg_mx = smx.tile([G, 1], F32, tag="negmx")
                nc.vector.tensor_reduce(
                    out=neg_mx, in_=sc_g[g],
                    axis=mybir.AxisListType.X, op=ALU.max, negate=True,
                )
                exp_t = smx.tile([G, 1 + L], F32, tag="exp")
                nc.scalar.activation(
                    out=exp_t, in_=sc_g[g],
                    func=ACTF.Exp, bias=neg_mx, scale=1.0,
                )
                ssum = smx.tile([G, 1], F32, tag="ssum")
                nc.vector.tensor_reduce(
                    out=ssum, in_=exp_t,
                    axis=mybir.AxisListType.X, op=ALU.add,
                )
                rsum = smx.tile([G, 1], F32, tag="rsum")
                nc.vector.reciprocal(rsum, ssum)
                pi_sb = smx.tile([G, 1 + L], BF16, tag="pi")
                nc.vector.tensor_scalar_mul(pi_sb, exp_t, rsum)

                pi_ps = ps_pi.tile(
                    [LC + 1, 2, G], BF16, tag="pi", name=f"pi_{g}"
                )
                nc.tensor.transpose(
                    pi_ps[:, 0, :], pi_sb[:, 0:LC + 1],
                    id_bf[0:G, 0:G],
                )
                nc.tensor.transpose(
                    pi_ps[0:LC, 1, :], pi_sb[:, LC + 1:1 + L],
                    id_bf[0:G, 0:G],
                )
                pi_of[g] = pi_ps

            def group_va(g):
                pi_ps = pi_of[g]
                for jv in range(G):
                    bv = G * g + jv
                    nc.vector.tensor_copy(
                        z_t[:, :, bv, bv:bv + 1], pi_ps[:, :, jv:jv + 1]
                    )
                    cq = conv_q[g]
                    for lc in range(2):
                        rows = LC + 1 if lc == 0 else LC
                        for h in range(2):
                            nc.tensor.matmul(
                                va_ps[h][:, :],
                                lhsT=z_t[0:rows, lc, bv, :],
                                rhs=cq[0:rows, jv, lc,
                                       512 * h:512 * h + 512],
                                start=(bv == 0 and lc == 0),
                                stop=(bv == S - 1 and lc == 1),
                            )

            ha_pair = None
            for b in range(S):
                q, j, jj = b // G, b % G, b % 2
                if jj == 0:
                    sum_pair = sump.tile([128, 2, CH, L], BF16, tag="sum")
                    ha_pair = hap.tile([128, 2, CH, LP], F8, tag="ha")
                nc.vector.tensor_tensor(
                    sum_pair[:, jj, :, :],
                    cfe_q[q][:, j, :, :],
                    hoe_t[:, :, b:b + 1].broadcast_to([128, CH, L]),
                    op=ALU.add,
                )
                nc.vector.tensor_copy(
                    ha_pair[:, jj, :, 0:1], ha0[:, :, b:b + 1]
                )
                if jj == 1:
                    nc.scalar.activation(
                        out=ha_pair[:, :, :, 1:1 + L],
                        in_=sum_pair,
                        func=ACTF.Tanh,
                    )
                    for bb in (b - 1, b):
                        jb = bb % G
                        for sp in range(4):
                            nc.tensor.matmul(
                                sc_g[q][:, :],
                                lhsT=waz_t[:, 2 * sp:2 * sp + 2, jb, :],
                                rhs=ha_pair[:, bb % 2,
                                            2 * sp:2 * sp + 2, 0:1 + L],
                                start=(jb == 0 and sp == 0),
                                stop=(jb == G - 1 and sp == 3),
                                perf_mode=DR,
                            )

                # group g's softmax chain emits ONE PAIR after its last
                # score matmul (the next pair's adds cover the
                # tanh->scores latency); its visAtt burst emits one pair
                # later still, so at the last pair the PE order is
                # [scores_p7, va_g2, piT_g3, va_g3] and g3's softmax
                # overlaps va_g2 instead of serializing after it.
                gv = None
                if jj == 1 and (b // 2) % 2 == 1 and b >= 7:
                    gv = (b - 7) // 4
                if gv is not None:
                    group_va(gv)
                g = None
                if jj == 1 and (b // 2) % 2 == 0 and b >= 4:
                    g = b // 4 - 1
                if b == S - 1:
                    g = NG - 1
                if g is not None:
                    group_chain(g)
                if b == S - 1:
                    group_va(NG - 1)

            # --- attn = visAtt + ho (transpose back); h dense --------------
            va_sb = acts.tile([S, D], BF16, tag="va_sb")
            nc.scalar.activation(
                out=va_sb[:, 0:512], in_=va_ps[0], func=ACTF.Copy,
            )
            nc.vector.tensor_copy(va_sb[:, 512:1024], va_ps[1])
            attn_ps = ps_va.tile([128, CH, S], BF16, tag="va", name="attn")
            for c in range(CH):
                nc.tensor.transpose(
                    attn_ps[:, c, :], va_sb[:, c * 128:(c + 1) * 128],
                    id_bf[0:S, 0:S],
                )
            attn = acts.tile([128, CH, S], BF16, tag="attn")
            nc.vector.tensor_add(attn, attn_ps, ho_t)
            h_sb = dense("h", attn, ACTF.Tanh, F32, w_h_c)
            nc.sync.dma_start(out=out_d[:, :, :], in_=h_sb)

    return nc


# ---------------------------------------------------------------------------

_NC_CACHE = {}


def _get_nc():
    if "nc" not in _NC_CACHE:
        nc = build_nc()
        nc.compile()
        _NC_CACHE["nc"] = nc
    return _NC_CACHE["nc"]


F8NP = mybir.dt.np(F8)
BFNP = mybir.dt.np(BF16)


def make_in_maps(inputs):
    def wpack(w, dt):
        # [128, CH, D]: w[p, k, o] = W[o, k*128+p]
        return np.ascontiguousarray(
            w.T.reshape(CH, 128, D).transpose(1, 0, 2).astype(dt)
        )

    shared = {
        "w_ho": wpack(np.asarray(inputs["W_ho"]), BFNP),
        "w_h": wpack(np.asarray(inputs["W_h"]), BFNP),
        "w_hoe": wpack(np.asarray(inputs["W_hoe"]), F8NP),
        "w_fr": wpack(np.asarray(inputs["W_fr"]), F8NP),
        "w_fre": wpack(np.asarray(inputs["W_fre"]), F8NP),
    }
    bias_row = np.stack(
        [np.asarray(inputs[f"b_{n}"]) for n in ("fr", "fre", "ho", "hoe", "h")]
    )  # [5, D]
    shared["bias_row"] = np.ascontiguousarray(bias_row[None].astype(BFNP))
    # wa_z[p, c, j, col] = wa[c*128+p] if col == j else 0
    wa_col = np.asarray(inputs["W_a"]).reshape(CH, 128).T.astype(F8NP)
    wa_z = np.zeros((128, CH, G, G), F8NP)
    for j in range(G):
        wa_z[:, :, j, j] = wa_col
    shared["wa_z"] = wa_z

    cfe_all = np.asarray(inputs["conv_feat_embed"])
    conv_all = np.asarray(inputs["conv_feat"])

    in_maps = []
    for i in range(N_CORES):
        sl = slice(i * S, (i + 1) * S)
        m = dict(shared)

        def xpack(x):
            # [128, CH, S]: x[p, k, b] = v[b, k*128+p]
            return np.ascontiguousarray(
                x.T.reshape(CH, 128, S).transpose(1, 0, 2).astype(BFNP)
            )

        m["xfr_T"] = xpack(np.asarray(inputs["fake_region"])[sl])
        m["xho_T"] = xpack(np.asarray(inputs["h_out"])[sl])

        # cfe8[p, b, s, l] = cfe[b, l, s*128+p]
        m["cfe8"] = np.ascontiguousarray(
            cfe_all[sl].transpose(2, 0, 1).reshape(CH, 128, S, L)
            .transpose(1, 2, 0, 3).astype(F8NP)
        )

        # conv8 lc0: row0 = l0 slot (runtime fr inject), rows 1..98 =
        # conv l 0..97; lc1: rows 0..97 = conv l 98..195
        conv8 = np.zeros((LC + 1, S, 2, D), F8NP)
        cs = conv_all[sl].astype(F8NP)          # [S, L, D]
        conv8[1:LC + 1, :, 0, :] = cs[:, 0:LC, :].transpose(1, 0, 2)
        conv8[0:LC, :, 1, :] = cs[:, LC:L, :].transpose(1, 0, 2)
        m["conv8"] = conv8
        in_maps.append(m)
    return in_maps


def run(inputs, trace=False, trace_kwargs=None):
    nc = _get_nc()
    in_maps = make_in_maps(inputs)
    res = run_bass_kernel_spmd(
        nc, in_maps, core_ids=list(range(N_CORES)), trace=trace,
        **(trace_kwargs or {}),
    )
    shards = [res.results[i]["out"] for i in range(N_CORES)]
    # out[p, c, b] = h[b, c*128+p]
    h = np.concatenate(
        [s.transpose(2, 1, 0).reshape(S, D) for s in shards], axis=0
    ).astype(np.float32)
    return h, res


def kernel(**inputs) -> np.ndarray:
    h, _ = run(inputs, trace=False)
    return h


if __name__ == "__main__":
    nc = build_nc()
    print(f"built ok: {len(nc.inst_map)} instructions")
